# revision 13
# baseline (speedup 1.0000x reference)
"""AdaptiveSkeletonLoss on 8 Trainium2 NeuronCores.

V2: DVE-pressure rewrite of the 402us baseline (which was 98.6% Vector-bound).

Layout: G=16 rows per partition; partition p = (sample s = p//32, row-block
b = p%32). Per-sample quantities live on partition ranges [32s, 32s+32), so
every per-sample column reduction becomes a TensorE matmul with a
block-diagonal ones stationary S4 [128,4] -> PSUM [4, 512] rows that are
DMA'd out and summed on the host.

Key changes vs v1:
- All fused scalar_tensor_tensor ops (always 1x on DVE) replaced by
  tensor_scalar (4x bf16) + tensor_tensor (2x bf16) pairs.
- All column-sum accumulations (1x cache-reduce on DVE) moved to TensorE
  ones-matmuls (engine was idle).
- Neighbor-count n built from a no-center W-pair-sum (sWnc) so the
  "n = boxsum - center" correction op disappears.
- Structural masks fold the `on` gate into the compare via n' = n * on:
  (n==v)&on <=> (n*on == v) for v in {1,2}, (n>2)&on <=> n*on > 2.
- Medial halo rows shrink 4x (2 rows x 16 words per partition).
- Phase order: dice counts (PE) early, then pack+medial, then structural,
  so PE count groups never sit in front of medial halo matmuls.
"""

import numpy as np

import concourse.bass as bass
import concourse.bacc as bacc
import concourse.mybir as mybir
from concourse.tile import TileContext
from concourse.bass_utils import run_bass_kernel_spmd

dt = mybir.dt
Alu = mybir.AluOpType
ActF = mybir.ActivationFunctionType

NCORES = 8
BS = 4            # samples per core
H = W = 512
P = 128           # partitions
G = 16            # rows per partition
NB = 32           # row-blocks per sample (= partitions per sample)
WP = W + 2        # padded row width
NW = 16           # int32 words per row (32 pixels each)
WPK = NW + 2      # padded packed row
NDIL = 9          # dilation levels
NPL = 4           # ripple-counter bit planes

# cnt DRAM rows (each [4, 512] f32, host sums the 512)
Q_SPG, Q_SP, Q_SG, Q_TP, Q_TG = 0, 1, 2, 3, 4
Q_IE, Q_IM, Q_IJ = 5, 6, 7
Q_PEC, Q_PMC, Q_PJC = 8, 9, 10
Q_GEC, Q_GMC, Q_GJC = 11, 12, 13
NQ = 14

# partials columns: (chain, plane) popcount partials; host separates samples
# by partition range [32s, 32s+32)
NCOL = 2 * NPL


def stt_i(eng, out, in0, scalar, in1, op0, op1, accum_out=None):
    """scalar_tensor_tensor with an int32-typed immediate (bitvec ops
    reject the float imm the stock helper emits)."""
    outs = [eng.lower_ap(out)]
    if accum_out is not None:
        outs.append(eng.lower_ap(accum_out))
    return eng.add_instruction(mybir.InstTensorScalarPtr(
        name=eng.bass.get_next_instruction_name(),
        is_scalar_tensor_tensor=True, op0=op0, op1=op1,
        ins=[eng.lower_ap(in0),
             mybir.ImmediateValue(dtype=mybir.dt.int32, value=scalar),
             eng.lower_ap(in1)],
        outs=outs))


def build_bass(do_dice=True, do_struct=True, do_medial=True):
    nc = bacc.Bacc()
    pred = nc.declare_dram_parameter("pred", [BS, H, W], dt.float32, isOutput=False)
    gt = nc.declare_dram_parameter("gt", [BS, H, W], dt.float32, isOutput=False)
    # block-diagonal shift matrices (sample-isolated):
    #   shdn: out[m] = in[m-1]  (down-shift), shup: out[m] = in[m+1]
    shup_d = nc.declare_dram_parameter("shup", [P, P], dt.bfloat16, isOutput=False)
    shdn_d = nc.declare_dram_parameter("shdn", [P, P], dt.bfloat16, isOutput=False)
    shupf_d = nc.declare_dram_parameter("shupf", [P, P], dt.float32, isOutput=False)
    shdnf_d = nc.declare_dram_parameter("shdnf", [P, P], dt.float32, isOutput=False)
    # per-sample ones selector [P, 4]: S4[k, m] = (k//32 == m)
    s4_d = nc.declare_dram_parameter("s4", [P, BS], dt.bfloat16, isOutput=False)
    partials_ext = nc.declare_dram_parameter("partials", [P, NCOL], dt.float32,
                                             isOutput=True)
    cnt_ext = nc.declare_dram_parameter("cnt", [NQ, BS, 512], dt.float32,
                                        isOutput=True)

    with TileContext(nc) as tc:
        with tc.tile_pool(name="pool", bufs=1) as pool:
            # ---------------- constants -------------------------------
            s4_t = pool.tile([P, BS], dt.bfloat16, tag="s4")
            shup_t = pool.tile([P, P], dt.bfloat16, tag="shup")
            shdn_t = pool.tile([P, P], dt.bfloat16, tag="shdn")
            shupf_t = pool.tile([P, P], dt.float32, tag="shupf")
            shdnf_t = pool.tile([P, P], dt.float32, tag="shdnf")
            nc.sync.dma_start(out=s4_t[:], in_=s4_d[:])
            nc.sync.dma_start(out=shup_t[:], in_=shup_d[:])
            nc.sync.dma_start(out=shdn_t[:], in_=shdn_d[:])
            nc.sync.dma_start(out=shupf_t[:], in_=shupf_d[:])
            nc.sync.dma_start(out=shdnf_t[:], in_=shdnf_d[:])

            partials = pool.tile([P, NCOL], dt.float32, tag="partials")
            nc.gpsimd.memset(partials[:], 0.0)

            # ---------------- inputs ----------------------------------
            pf = pool.tile([P, G, W], dt.float32, tag="pf")
            gf = pool.tile([P, G, W], dt.float32, tag="gf")
            # chunked loads so compute can start before the full 8MB lands
            pr_ap = pred[:].rearrange("s (b r) w -> (s b) r w", r=G)
            gt_ap = gt[:].rearrange("s (b r) w -> (s b) r w", r=G)
            HC = G // 2
            for ci in range(2):
                nc.sync.dma_start(out=pf[:, HC * ci:HC * (ci + 1), :],
                                  in_=pr_ap[:, HC * ci:HC * (ci + 1), :])
                nc.sync.dma_start(out=gf[:, HC * ci:HC * (ci + 1), :],
                                  in_=gt_ap[:, HC * ci:HC * (ci + 1), :])

            def padded(tag):
                t = pool.tile([P, G, WP], dt.bfloat16, tag=tag)
                nc.gpsimd.memset(t[:], 0.0)
                return t

            pbf = padded("pbf")   # raw pred in bf16, zero-padded cols
            gbf = padded("gbf")
            pb = pool.tile([P, G, W], dt.bfloat16, tag="pb")   # pred > 0.5
            gb = pool.tile([P, G, W], dt.bfloat16, tag="gb")
            prod = pool.tile([P, G, W], dt.bfloat16, tag="prod")

            # casts on ScalarE (idle engine), chunked to chase the DMAs
            for ci in range(2):
                sl = slice(HC * ci, HC * (ci + 1))
                nc.scalar.activation(out=pbf[:, sl, 1:1 + W], in_=pf[:, sl],
                                     func=ActF.Copy)
                nc.scalar.activation(out=gbf[:, sl, 1:1 + W], in_=gf[:, sl],
                                     func=ActF.Copy)
            # binarize from bf16 (4x TS); ~0.1% of pred pixels sit on the
            # bf16 rounding knife-edge at 0.5 -- well inside tolerance
            for ci in range(2):
                sl = slice(HC * ci, HC * (ci + 1))
                nc.vector.tensor_scalar(out=pb[:, sl], in0=pbf[:, sl, 1:1 + W],
                                        scalar1=0.5, scalar2=None, op0=Alu.is_gt)
            nc.vector.tensor_scalar(out=gb[:], in0=gbf[:, :, 1:1 + W],
                                    scalar1=0.5, scalar2=None, op0=Alu.is_gt)
            nc.vector.tensor_tensor(out=prod[:], in0=pbf[:, :, 1:1 + W],
                                    in1=gbf[:, :, 1:1 + W], op=Alu.mult)

            # ---------------- PSUM pools ------------------------------
            cnt_cm = tc.tile_pool(name="cnt_ps", bufs=1, space="PSUM")
            cnt_ps = cnt_cm.__enter__()
            # 4 banks x 4 slots (base partitions 0/32/64/96)
            cbank = []
            for i in range(5):
                cb = cnt_ps.tile([P, 512], dt.float32, tag=f"cb{i}", name=f"cb{i}")
                cbank.append(cb)
            # SBUF bounce for psum->dram (DMA cannot read PSUM)
            cnt_sb = pool.tile([P, 1, 512], dt.float32, tag="cnt_sb")

            def count_group(q, src_ap_fn):
                """Accumulate per-sample column sums of a [P, G, W] bf16 tile
                into psum slot q; ScalarE-copy to SBUF; DMA to cnt[q]."""
                bank = cbank[q % 5]
                base = 32 * (q // 5)
                out_ap = bank[base:base + BS, :]
                for r in range(G):
                    nc.tensor.matmul(out_ap, s4_t[:], src_ap_fn(r),
                                     start=(r == 0), stop=(r == G - 1))
                sb = cnt_sb[base:base + BS, 0, :]
                nc.scalar.copy(out=sb, in_=out_ap)
                nc.sync.dma_start(out=cnt_ext[q], in_=sb)

            # ---------------- dice counts (PE, early) -----------------
            if do_dice:
                count_group(Q_SPG, lambda r: prod[:, r, :])
                count_group(Q_SP, lambda r: pbf[:, r, 1:1 + W])
                count_group(Q_SG, lambda r: gbf[:, r, 1:1 + W])
                count_group(Q_TP, lambda r: pb[:, r, :])
                count_group(Q_TG, lambda r: gb[:, r, :])

            work_cm = tc.tile_pool(name="work_ps", bufs=1, space="PSUM")
            work_ps = work_cm.__enter__()

            # ---------------- pack pb/gb -> pkP/pkG -------------------
            def packed_tile(tag, rows=G):
                t = pool.tile([P, rows, WPK], dt.int32, tag=tag)
                nc.gpsimd.memset(t[:], 0)
                return t

            pkG = packed_tile("pkG")
            pkP = packed_tile("pkP")
            pkA = packed_tile("pkA")
            pkB = packed_tile("pkB")
            tWp = packed_tile("tWp", rows=G + 2)

            # reuse dead early-phase tags: pf/gf are consumed by the casts,
            # binarize and dice count groups before the pack starts
            pt1 = pool.tile([P, G, 256], dt.float32, tag="pf")
            pt2 = pool.tile([P, G, 128], dt.float32, tag="gf")
            gi = pool.tile([P, G, 32], dt.int32, tag="su")

            for img, dst in ((pb, pkP), (gb, pkG)) if do_medial else []:
                nc.vector.scalar_tensor_tensor(
                    out=pt1[:], in0=img[:, :, 1:W:2], scalar=2.0,
                    in1=img[:, :, 0:W:2], op0=Alu.mult, op1=Alu.add)
                nc.vector.scalar_tensor_tensor(
                    out=pt2[:], in0=pt1[:, :, 1:256:2], scalar=4.0,
                    in1=pt1[:, :, 0:256:2], op0=Alu.mult, op1=Alu.add)
                nc.vector.scalar_tensor_tensor(
                    out=pt1[:, :, 0:64], in0=pt2[:, :, 1:128:2], scalar=16.0,
                    in1=pt2[:, :, 0:128:2], op0=Alu.mult, op1=Alu.add)
                nc.vector.scalar_tensor_tensor(
                    out=pt2[:, :, 0:32], in0=pt1[:, :, 1:64:2], scalar=256.0,
                    in1=pt1[:, :, 0:64:2], op0=Alu.mult, op1=Alu.add)
                nc.vector.tensor_copy(gi[:], pt2[:, :, 0:32])
                stt_i(nc.vector, dst[:, :, 1:1 + NW],
                      gi[:, :, 1:32:2], 16, gi[:, :, 0:32:2],
                      Alu.logical_shift_left, Alu.bitwise_or)

            # ---------------- medial (bit-packed dilation) ------------
            # Count planes: S(x) = #levels d with x in D_d = 10 - first-level.
            # D_d is monotone, so newly-found masks n_d = D_d ^ D_{d-1} are
            # disjoint and the planes of S are pure ORs of n_d with weight
            # bits of (10-d). Counter updates are placed inside the NEXT
            # level's PE halo round-trip to fill the DVE bubble.
            c4 = pool.tile([P, NPL, G, NW], dt.int32, tag="c4")
            nd = pool.tile([P, G, NW], dt.int32, tag="nd")
            eint = pool.tile([P, 2, 2, NW], dt.int32, tag="eint")
            ef = pool.tile([P, 2, 2, NW], dt.float32, tag="ef")
            ei2 = pool.tile([P, 2, 2, NW], dt.int32, tag="ei2")
            su = pool.tile([P, NPL, 2, G, NW], dt.int32, tag="su")
            sv = pool.tile([P, NPL, 2, G, NW], dt.int32, tag="sv")

            def dataw(t):
                return t[:, 0:G, 1:1 + NW]

            twd = tWp[:, 1:1 + G, 1:1 + NW]
            # plane bits of weight (10-d) for newly-found-at-level-d pixels
            WPLANES = {2: (3,), 3: (0, 1, 2), 4: (1, 2), 5: (0, 2),
                       6: (2,), 7: (0, 1), 8: (1,), 9: (0,)}

            def dt_of(d, seed):
                if d <= 0:
                    return seed
                return pkA if d % 2 == 1 else pkB

            def counter_update(d, seed):
                dd = dataw(dt_of(d, seed))
                if d == 1:
                    # weight 9 (= planes 0,3) covers both seed and level-1
                    nc.vector.tensor_copy(c4[:, 0], dd)
                    nc.vector.tensor_copy(c4[:, 3], dd)
                    return
                dp = dataw(dt_of(d - 1, seed))
                nc.vector.tensor_tensor(out=nd[:], in0=dd, in1=dp,
                                        op=Alu.bitwise_xor)
                for k in WPLANES[d]:
                    if d == 3 and k in (1, 2):
                        # first touch of planes 1/2 each chain: overwrite
                        # (no memset between chains)
                        nc.vector.tensor_copy(c4[:, k], nd[:])
                    else:
                        nc.vector.tensor_tensor(out=c4[:, k], in0=c4[:, k],
                                                in1=nd[:], op=Alu.bitwise_or)

            for chain, (seed, tmask) in enumerate(
                    ((pkG, pkP), (pkP, pkG))) if do_medial else []:
                cur = seed
                for d in range(1, NDIL + 1):
                    nxt = pkA if (d % 2 == 1) else pkB
                    cw = dataw(cur)
                    # W dilation (4 fused shift-or ops; pads give zero carries)
                    stt_i(nc.vector, twd, cw, 1, cw,
                          Alu.logical_shift_left, Alu.bitwise_or)
                    stt_i(nc.vector, twd, cw, 1, twd,
                          Alu.logical_shift_right, Alu.bitwise_or)
                    stt_i(nc.vector, twd, cur[:, 0:G, 0:NW], 31, twd,
                          Alu.logical_shift_right, Alu.bitwise_or)
                    stt_i(nc.vector, twd, cur[:, 0:G, 2:2 + NW], 31, twd,
                          Alu.logical_shift_left, Alu.bitwise_or)
                    # halo transport: slot 0 <- row 15 of p-1 (shdn),
                    # slot 17 <- row 0 of p+1 (shup); exact 16-bit halves
                    # through f32 shift matmuls.
                    for di, rr in ((0, G), (1, 1)):  # src slots: 16 / 1
                        srcw = tWp[:, rr, 1:1 + NW]
                        nc.vector.tensor_scalar(
                            out=eint[:, di, 0], in0=srcw, scalar1=0xFFFF,
                            scalar2=None, op0=Alu.bitwise_and)
                        nc.vector.tensor_scalar(
                            out=eint[:, di, 1], in0=srcw, scalar1=16,
                            scalar2=None, op0=Alu.logical_shift_right)
                    nc.vector.tensor_copy(
                        ef[:].rearrange("p a b c -> p (a b c)"),
                        eint[:].rearrange("p a b c -> p (a b c)"))
                    pet = work_ps.tile([P, 2, 2 * NW], dt.float32, tag="pet")
                    nc.tensor.matmul(pet[:, 0], shdnf_t[:],
                                     ef[:, 0].rearrange("p a b -> p (a b)"),
                                     start=True, stop=True)
                    nc.tensor.matmul(pet[:, 1], shupf_t[:],
                                     ef[:, 1].rearrange("p a b -> p (a b)"),
                                     start=True, stop=True)
                    # fill the PE round-trip with the previous level's counter
                    if d >= 2:
                        counter_update(d - 1, seed)
                    nc.vector.tensor_copy(
                        ei2[:].rearrange("p a b c -> p (a b c)"),
                        pet[:].rearrange("p a b -> p (a b)"))
                    stt_i(nc.vector, tWp[:, 0, 1:1 + NW],
                          ei2[:, 0, 1], 16, ei2[:, 0, 0],
                          Alu.logical_shift_left, Alu.bitwise_or)
                    stt_i(nc.vector, tWp[:, G + 1, 1:1 + NW],
                          ei2[:, 1, 1], 16, ei2[:, 1, 0],
                          Alu.logical_shift_left, Alu.bitwise_or)
                    # V dilation: OR over the 3-slot window
                    nw_ = dataw(nxt)
                    nc.vector.tensor_tensor(
                        out=nw_, in0=tWp[:, 0:G, 1:1 + NW],
                        in1=tWp[:, 1:1 + G, 1:1 + NW], op=Alu.bitwise_or)
                    nc.vector.tensor_tensor(
                        out=nw_, in0=nw_,
                        in1=tWp[:, 2:2 + G, 1:1 + NW], op=Alu.bitwise_or)
                    cur = nxt
                counter_update(NDIL, seed)
                # ---- extraction: batched popcount(c_k & t) over all planes
                tm = dataw(tmask)
                mhi = sv[:, 0, 0]   # scratch: sv not yet written
                nc.vector.tensor_scalar(out=mhi, in0=tm, scalar1=16,
                                        scalar2=None,
                                        op0=Alu.logical_shift_right)
                for k in range(NPL):
                    stt_i(nc.vector, su[:, k, 0], c4[:, k], 0xFFFF, tm,
                          Alu.bitwise_and, Alu.bitwise_and)
                    stt_i(nc.vector, su[:, k, 1], c4[:, k], 16, mhi,
                          Alu.logical_shift_right, Alu.bitwise_and)
                sur = su[:].rearrange("p a b c d -> p (a b c) d")
                svr = sv[:].rearrange("p a b c d -> p (a b c) d")
                # 16-bit SWAR popcount (exact through f32-valued int ALU)
                nc.vector.tensor_scalar(out=svr, in0=sur, scalar1=1,
                                        scalar2=0x5555,
                                        op0=Alu.logical_shift_right,
                                        op1=Alu.bitwise_and)
                nc.vector.tensor_tensor(out=sur, in0=sur, in1=svr,
                                        op=Alu.subtract)
                nc.vector.tensor_scalar(out=svr, in0=sur, scalar1=2,
                                        scalar2=0x3333,
                                        op0=Alu.logical_shift_right,
                                        op1=Alu.bitwise_and)
                nc.vector.tensor_scalar(out=sur, in0=sur, scalar1=0x3333,
                                        scalar2=None, op0=Alu.bitwise_and)
                nc.vector.tensor_tensor(out=sur, in0=sur, in1=svr, op=Alu.add)
                nc.vector.tensor_scalar(out=svr, in0=sur, scalar1=4,
                                        scalar2=None,
                                        op0=Alu.logical_shift_right)
                nc.vector.tensor_tensor(out=sur, in0=sur, in1=svr, op=Alu.add)
                nc.vector.tensor_scalar(out=sur, in0=sur, scalar1=0x0F0F,
                                        scalar2=None, op0=Alu.bitwise_and)
                # multiply-popcount finish: (v * 257) >> 8 & 0x1F
                nc.vector.tensor_scalar(out=sur, in0=sur, scalar1=257,
                                        scalar2=None, op0=Alu.mult)
                nc.vector.tensor_scalar(out=sur, in0=sur, scalar1=8,
                                        scalar2=0x1F,
                                        op0=Alu.logical_shift_right,
                                        op1=Alu.bitwise_and)
                nc.vector.tensor_reduce(
                    out=partials[:, chain * NPL:(chain + 1) * NPL],
                    in_=su[:].rearrange("p a b c d -> p a (b c d)"),
                    axis=mybir.AxisListType.X, op=Alu.add)

            # ---------------- structural ------------------------------
            sWnc = padded("sWnc")   # x[j-1] + x[j+1]
            sW = padded("sW")       # 3-col sum
            nP = pool.tile([P, G, W], dt.bfloat16, tag="pf")
            nG = pool.tile([P, G, W], dt.bfloat16, tag="gf")

            for src, xb, ndst in (((pbf, pb, nP), (gbf, gb, nG))
                                  if do_struct else []):
                nc.vector.tensor_tensor(
                    out=sWnc[:, :, 1:1 + W], in0=src[:, :, 0:W],
                    in1=src[:, :, 2:2 + W], op=Alu.add)
                nc.vector.tensor_tensor(
                    out=sW[:, :, 1:1 + W], in0=sWnc[:, :, 1:1 + W],
                    in1=src[:, :, 1:1 + W], op=Alu.add)
                # halo rows via PE (block-diag shift matmuls)
                dn_ps = work_ps.tile([P, W], dt.float32, tag="dn")
                up_ps = work_ps.tile([P, W], dt.float32, tag="up")
                nc.tensor.matmul(dn_ps[:], shdn_t[:], sW[:, G - 1, 1:1 + W],
                                 start=True, stop=True)
                nc.tensor.matmul(up_ps[:], shup_t[:], sW[:, 0, 1:1 + W],
                                 start=True, stop=True)
                # n = row-above + (no-center row) + row-below
                nc.vector.tensor_tensor(
                    out=ndst[:, 1:G, :], in0=sW[:, 0:G - 1, 1:1 + W],
                    in1=sWnc[:, 1:G, 1:1 + W], op=Alu.add)
                nc.vector.tensor_tensor(
                    out=ndst[:, 0, :], in0=dn_ps[:],
                    in1=sWnc[:, 0, 1:1 + W], op=Alu.add)
                nc.vector.tensor_tensor(
                    out=ndst[:, 0:G - 1, :], in0=ndst[:, 0:G - 1, :],
                    in1=sW[:, 1:G, 1:1 + W], op=Alu.add)
                nc.vector.tensor_tensor(
                    out=ndst[:, G - 1, :], in0=ndst[:, G - 1, :],
                    in1=up_ps[:], op=Alu.add)
                # fold the on-mask: n' = n * (x > 0.5)
                nc.vector.tensor_tensor(out=ndst[:], in0=ndst[:], in1=xb[:],
                                        op=Alu.mult)

            # masks: (n'==1), (n'==2), (n'>2); intersections; PE counts
            mp = pool.tile([P, G, W], dt.bfloat16, tag="pbf")
            mg = pool.tile([P, G, W], dt.bfloat16, tag="gbf")
            inter = pool.tile([P, G, W], dt.bfloat16, tag="prod")

            for (cmp_op, val, q_i, q_pc, q_gc) in (() if not do_struct else (
                    (Alu.is_equal, 1.0, Q_IE, Q_PEC, Q_GEC),
                    (Alu.is_equal, 2.0, Q_IM, Q_PMC, Q_GMC),
                    (Alu.is_gt, 2.0, Q_IJ, Q_PJC, Q_GJC))):
                nc.vector.tensor_scalar(out=mp[:], in0=nP[:], scalar1=val,
                                        scalar2=None, op0=cmp_op)
                nc.vector.tensor_scalar(out=mg[:], in0=nG[:], scalar1=val,
                                        scalar2=None, op0=cmp_op)
                nc.vector.tensor_tensor(out=inter[:], in0=mp[:], in1=mg[:],
                                        op=Alu.mult)
                count_group(q_pc, lambda r: mp[:, r, :])
                count_group(q_gc, lambda r: mg[:, r, :])
                count_group(q_i, lambda r: inter[:, r, :])

            work_cm.__exit__(None, None, None)
            cnt_cm.__exit__(None, None, None)
            nc.sync.dma_start(out=partials_ext[:], in_=partials[:])

    return nc


_NC_CACHE = None


def _get_nc():
    global _NC_CACHE
    if _NC_CACHE is None:
        import os
        nc = build_bass(do_dice=os.environ.get("K_DICE", "1") == "1",
                        do_struct=os.environ.get("K_STRUCT", "1") == "1",
                        do_medial=os.environ.get("K_MEDIAL", "1") == "1")
        nc.finalize()
        _NC_CACHE = nc
    return _NC_CACHE


def epilogue(partials_by_sample):
    """partials_by_sample [B, 16] -> final scalar (float32)."""
    q = partials_by_sample.astype(np.float64)
    s_pg, s_p, s_g, t_p, t_g = q[:, 0], q[:, 1], q[:, 2], q[:, 3], q[:, 4]
    ie, im, ij = q[:, 5], q[:, 6], q[:, 7]
    pe_c, pm_c, pj_c = q[:, 8], q[:, 9], q[:, 10]
    ge_c, gm_c, gj_c = q[:, 11], q[:, 12], q[:, 13]
    A_p2g, A_g2p = q[:, 14], q[:, 15]

    dice = (2 * s_pg + 1) / (s_p + s_g + 1)
    dice_loss = 1 - dice.mean()

    e_iou = (ie + 1) / (pe_c + ge_c - ie + 1)
    m_iou = (im + 1) / (pm_c + gm_c - im + 1)
    j_iou = (ij + 1) / (pj_c + gj_c - ij + 1)
    total = ge_c + gj_c + gm_c + 1
    struct = 1 - ((ge_c / total) * e_iou + (gj_c / total) * j_iou
                  + (gm_c / total) * m_iou)
    structural_loss = struct.mean()

    p2g = (10 * t_p - A_p2g) / (t_p + 1)
    g2p = (10 * t_g - A_g2p) / (t_g + 1)
    medial_loss = (((p2g + g2p) / 2) / 10).mean()

    avg = (dice_loss + structural_loss + medial_loss) / 3
    out = (dice_loss / (dice_loss + 1) * avg
           + structural_loss / (structural_loss + 1) * avg
           + medial_loss / (medial_loss + 1) * avg)
    return np.float32(out)


def run_device(pred_skel, gt_skel, trace=False):
    """Returns (partials [B,16] np.float64, bass results object)."""
    nc = _get_nc()
    pred = np.ascontiguousarray(np.asarray(pred_skel, np.float32)[:, 0])
    gtv = np.ascontiguousarray(np.asarray(gt_skel, np.float32)[:, 0])
    import ml_dtypes
    # block-diagonal shift matrices: zero at sample boundaries
    shdn = np.eye(P, k=1, dtype=np.float32)
    shup = np.eye(P, k=-1, dtype=np.float32)
    for sb in range(NB, P, NB):
        shdn[sb - 1, sb] = 0.0
        shup[sb, sb - 1] = 0.0
    s4 = np.zeros((P, BS), dtype=ml_dtypes.bfloat16)
    for k in range(P):
        s4[k, k // NB] = 1.0
    in_maps = [
        {"pred": np.ascontiguousarray(pred[c * BS:(c + 1) * BS]),
         "gt": np.ascontiguousarray(gtv[c * BS:(c + 1) * BS]),
         "shup": shup.astype(ml_dtypes.bfloat16),
         "shdn": shdn.astype(ml_dtypes.bfloat16),
         "shupf": shup, "shdnf": shdn, "s4": s4}
        for c in range(NCORES)
    ]
    res = run_bass_kernel_spmd(nc, in_maps, core_ids=list(range(NCORES)),
                               trace=trace)
    parts = []
    w = np.array([1.0, 2.0, 4.0, 8.0])
    for c in range(NCORES):
        r = res.results[c]
        cnt = r["cnt"].astype(np.float64)          # [NQ, BS, 512]
        pp = r["partials"].astype(np.float64)      # [P, NCOL]
        q = np.zeros((BS, 16))
        q[:, :NQ] = cnt.sum(axis=2).T              # [BS, NQ]
        for s in range(BS):
            sl = pp[NB * s:NB * (s + 1)]           # [32, NCOL]
            q[s, 14] = (sl[:, 0:NPL].sum(axis=0) * w).sum()
            q[s, 15] = (sl[:, NPL:2 * NPL].sum(axis=0) * w).sum()
        parts.append(q)
    return np.concatenate(parts, axis=0), res


def kernel(pred_skel, gt_skel):
    partials, _ = run_device(pred_skel, gt_skel, trace=False)
    return epilogue(partials)
